# revision 18
# baseline (speedup 1.0000x reference)
"""EnhancedMultiHeadAttention on 8 Trainium2 NeuronCores (Bass/Tile).

Sharding: core c -> batch b = c//4, head group g = c%4 (4 heads of 16).
Per core, everything is computed in "transposed" layout [feature, token]:
  - LayerNorm stats via ones-matmul column sums of x and x^2 (PE), then
    normalize xT in place with broadcast mu/rstd rows (K=1 ones matmuls).
  - Fused projections q/k/gate in [feat, tok] layout; v in [tok, feat]
    layout (lhsT = zT) augmented with a ones column per head so the
    attention AV matmul also produces the softmax denominator.
  - Scores^T via lhsT=kT slice, rhs=qT slice (K=HD=64, head pairs packed
    into PE row groups 0-63/64-127); softmax over k is a plain exp (no
    max subtraction; scores are provably small for this model) and the
    denominator lands in ctx PSUM row 64 via the V ones column.
  - ctx rows are normalized by 1/denominator (broadcast via K=1 matmul),
    AllGather'd across the 4 cores of the batch group per q-block, then
    out = ctx_all @ w_out[:, cols] + b_out, gated and residual-added.
All LayerNorm gamma/beta and the 1/sqrt(HD) scale are folded into the
weights/biases on the host. sigmoid and rsqrt are computed via exp/ln so
the whole kernel uses one ACT table set (natural_log_exp_and_others).
"""

import contextlib
import os

import numpy as np

import jax

jax.config.update("jax_compilation_cache_dir", os.path.expanduser("~/.bass_jax_cache"))
jax.config.update("jax_persistent_cache_min_compile_time_secs", 0.0)
jax.config.update("jax_persistent_cache_min_entry_size_bytes", 0)

import concourse.bass as bass
import concourse.bacc as bacc
import concourse.tile as tile
from concourse import mybir
from concourse.bass_utils import run_bass_kernel_spmd

B, S, D, H, HD = 2, 2048, 1024, 16, 64
NCORES = 8
GROUPS = [[0, 1, 2, 3], [4, 5, 6, 7]]
TB = 512  # token block
NB = S // TB  # 4
DC = D // 128  # 8 K-chunks
FH = 4  # heads per core
FQ = FH * HD  # 256 feature columns per core
FP = mybir.dt.float32
FR = mybir.dt.float32r  # TF32-like: 4x matmul throughput vs fp32
F16 = mybir.dt.float16  # halves AllGather bytes; ~5e-4 elementwise rounding
AF = mybir.ActivationFunctionType
EPS = 1e-5

_NC_CACHE = {}


def _bcast_ap(handle, parts):
    ap = handle.ap()
    return bass.AP(
        tensor=ap.tensor,
        offset=ap.offset,
        ap=[[0, parts]] + [list(p) for p in ap.ap],
    )


def _body(tc, t):
    nc = tc.nc
    stack = contextlib.ExitStack()
    stack.enter_context(
        nc.allow_low_precision(reason="fp32r/fp16 rounding is intentional; all matmul accumulation stays fp32 in PSUM")
    )
    pool = lambda name, bufs, space="SBUF": stack.enter_context(
        tc.tile_pool(name=name, bufs=bufs, space=space)
    )

    consts = pool("consts", 1)
    singles = pool("singles", 1)
    dramp = pool("dramp", 2, "DRAM")

    # PSUM pools (8 banks): sc 2x[128,1024]=4 | ctx 2 | mmA 2
    ps_sc = pool("ps_sc", 2, "PSUM")    # scores (double-wide) + stats x^2
    ps_ctx = pool("ps_ctx", 1, "PSUM")  # 2 tags: ctx accumulators (head pair)
    ps_mmA = pool("ps_mmA", 2, "PSUM")  # phase A rotation + out-proj + denom bcast

    pA_x = pool("pA_x", 2)      # [128, DC, TB] x block     32KB
    pA_sq = pool("pA_sq", 2)    # [128, TB] squares          4KB
    pA_rows = pool("pA_rows", 2)  # [1, TB] msq/var/lnv      ~8KB
    pA_ge = pool("pA_ge", 1)    # [128, TB] gate tmp         2KB
    pA_vt = pool("pA_vt", 2)    # [128, FQ] v evac tmp       2KB
    pB_pr = pool("pB_pr", 3)    # [128, 2*TB] f16 probs      6KB
    pB_rows = pool("pB_rows", 2)  # [1, TB] recip denom      4KB
    pB_bcs = pool("pB_bcs", 2)  # [64, TB] denom bcast       4KB
    pB_ctxT = pool("pB_ctxT", 2)  # [64, FH, TB] f16 ctx     8KB
    pC_ca = pool("pC_ca", 2)    # [128, TB] f16 ctx_all      2KB
    pC_xr = pool("pC_xr", 1)    # [128, 2, TB] residual      4KB
    pC_osb = pool("pC_osb", 2)  # [128, TB] out staging      4KB

    # constants (fp32r tiles can't be memset directly; stage fp32 + DVE copy)
    onesf_col = consts.tile([128, 1], FP)
    nc.vector.memset(onesf_col, 1.0)
    onesf_row = consts.tile([1, 128], FP)
    nc.vector.memset(onesf_row, 1.0)
    ones_col = consts.tile([128, 1], FR)
    nc.vector.tensor_copy(out=ones_col, in_=onesf_col)
    ones_row = consts.tile([1, 128], FR)
    nc.vector.tensor_copy(out=ones_row, in_=onesf_row)
    eps_t = consts.tile([1, 1], FP)
    nc.vector.memset(eps_t, EPS)

    # resident weights
    wqkg_sb = singles.tile([128, DC, 3 * FQ], FR)
    nc.gpsimd.dma_start(out=wqkg_sb, in_=t["wqkg"].ap().rearrange("(d p) f -> p d f", p=128))
    wv_sb = singles.tile([128, DC, FQ], FR)
    nc.gpsimd.dma_start(out=wv_sb, in_=t["wv"].ap().rearrange("(d p) f -> p d f", p=128))
    wout_sb = singles.tile([128, DC, FQ], F16)
    nc.gpsimd.dma_start(out=wout_sb, in_=t["wout"].ap().rearrange("(d p) f -> p d f", p=128))
    ncs_sb = singles.tile([1, 3 * FQ], FR)
    nc.gpsimd.dma_start(out=ncs_sb, in_=t["ncs"].ap().rearrange("(o f) -> o f", o=1))
    ncsv_sb = singles.tile([1, FQ], FR)
    nc.gpsimd.dma_start(out=ncsv_sb, in_=t["ncsv"].ap().rearrange("(o f) -> o f", o=1))
    bqkg_sb = singles.tile([128, 6], FP)
    nc.sync.dma_start(out=bqkg_sb, in_=t["bqkg"].ap().rearrange("(m p) -> p m", p=128))
    bout_sb = singles.tile([128, 2], FP)
    nc.sync.dma_start(out=bout_sb, in_=t["bout"].ap().rearrange("(m p) -> p m", p=128))
    bv_sb = singles.tile([128, FQ], FP)
    nc.sync.dma_start(out=bv_sb, in_=_bcast_ap(t["bv"], 128))

    # resident activations + per-block LN stats
    qT = singles.tile([128, 2, S], FR)
    kT = singles.tile([128, 2, S], FR)
    gT = singles.tile([128, 2, S], FP)
    va = singles.tile([128, S // 128, FH, HD + 1], F16)  # [k-part, kc, h, 65]
    for _kc in range(S // 128):
        for _h in range(FH):
            nc.vector.tensor_copy(out=va[:, _kc, _h, HD:HD + 1], in_=onesf_col)
    pA_mu = pool("pA_mu", 2)    # [1, TB] FR mean rows (A0(i) -> A1(i))
    pA_rsb = pool("pA_rsb", 2)  # [128, TB] rstd broadcast
    pA_rsc = pool("pA_rsc", 2)  # [128, 4] rstd columns
    mus, rsbs, rscs = {}, {}, {}

    xT_r = t["xT"].ap().rearrange("(d p) tk -> p d tk", p=128)
    xres_r = t["xres"].ap().rearrange("(m p) tk -> p m tk", p=128)

    xblks = {}

    # ---------------- Phase A0: LN stats for one token block --------------
    def phase_a0(i):
        tb = slice(i * TB, (i + 1) * TB)
        xblk = pA_x.tile([128, DC, TB], FR, tag="xblk", name=f"xblk{i}")
        nc.gpsimd.dma_start(out=xblk, in_=xT_r[:, :, tb])
        xblks[i] = xblk

        psx = ps_mmA.tile([1, TB], FP, tag="mmA", name=f"psx{i}")
        for d in range(DC):
            nc.tensor.matmul(
                out=psx, lhsT=ones_col, rhs=xblk[:, d, :],
                start=(d == 0), stop=(d == DC - 1),
            )
        pssq = ps_sc.tile([1, TB], FP, tag="sc", name=f"pssq{i}")
        for d in range(DC):
            xsq = pA_sq.tile([128, TB], FR, tag="xsq", name=f"xsq{i}_{d}")
            nc.vector.tensor_mul(out=xsq, in0=xblk[:, d, :], in1=xblk[:, d, :])
            nc.tensor.matmul(
                out=pssq, lhsT=ones_col, rhs=xsq,
                start=(d == 0), stop=(d == DC - 1),
            )
        mu = pA_mu.tile([1, TB], FR, tag="mu", name=f"mu{i}")
        mus[i] = mu
        nc.scalar.activation(out=mu, in_=psx, func=AF.Copy, scale=1.0 / D)
        msq = pA_rows.tile([1, TB], FP, tag="msq", name=f"msq{i}")
        nc.scalar.activation(out=msq, in_=pssq, func=AF.Copy, scale=1.0 / D)
        var = pA_rows.tile([1, TB], FP, tag="var", name=f"var{i}")
        nc.vector.tensor_mul(out=var, in0=mu, in1=mu)
        nc.vector.tensor_sub(out=var, in0=msq, in1=var)
        # rstd = exp(-0.5 * ln(var + eps))  (keeps everything in one ACT table set)
        lnv = pA_rows.tile([1, TB], FP, tag="lnv", name=f"lnv{i}")
        nc.scalar.activation(out=lnv, in_=var, func=AF.Ln, bias=eps_t[0:1, :])
        rstd = pA_rows.tile([1, TB], FR, tag="rstd", name=f"rstd{i}")
        nc.scalar.activation(out=rstd, in_=lnv, func=AF.Exp, scale=-0.5)
        # broadcast rstd to all partitions (row) and as per-token columns
        rs_b = pA_rsb.tile([128, TB], FP, tag="rs_b", name=f"rsb{i}")
        rsbs[i] = rs_b
        pbc2 = ps_mmA.tile([128, TB], FP, tag="mmA", name=f"pbcrs{i}")
        nc.tensor.matmul(out=pbc2, lhsT=ones_row, rhs=rstd, start=True, stop=True)
        nc.scalar.activation(out=rs_b, in_=pbc2, func=AF.Copy)
        rsc = pA_rsc.tile([128, 4], FR, tag="rsc", name=f"rsc{i}")
        rscs[i] = rsc
        for a in range(4):
            nc.sync.dma_start(
                out=rsc[:, a:a + 1], in_=rstd[0:1, a * 128:(a + 1) * 128]
            )

    # ---------------- Phase A1: projections for one token block -----------
    def phase_a1(i):
        tb = slice(i * TB, (i + 1) * TB)
        xblk = xblks.pop(i)
        mu = mus.pop(i)
        rs_b = rsbs.pop(i)
        rsc = rscs.pop(i)
        # q/k/gate projections on RAW x; mean subtraction folded in as a
        # rank-1 correction (ncs = -colsum(W)); rstd applied at evacuation:
        #   W^T((x-mu)rstd) = rstd * (W^T x + ncs * mu)
        for m in range(6):
            pqk = ps_mmA.tile([128, TB], FP, tag="mmA", name=f"pqk{i}_{m}")
            for d in range(DC):
                nc.tensor.matmul(
                    out=pqk,
                    lhsT=wqkg_sb[:, d, m * 128:(m + 1) * 128],
                    rhs=xblk[:, d, :],
                    start=(d == 0), stop=False,
                )
            nc.tensor.matmul(
                out=pqk, lhsT=ncs_sb[0:1, m * 128:(m + 1) * 128], rhs=mu,
                start=False, stop=True,
            )
            if m < 4:
                dst = qT[:, m, tb] if m < 2 else kT[:, m - 2, tb]
                nc.vector.tensor_mul(out=dst, in0=pqk, in1=rs_b)
                nc.vector.tensor_scalar_add(
                    out=dst, in0=dst, scalar1=bqkg_sb[:, m:m + 1]
                )
            else:
                # gate = sigmoid(u + b) = 1 / (1 + exp(-u - b)); bias slot holds -b
                ge = pA_ge.tile([128, TB], FP, tag="ge", name=f"ge{i}_{m}")
                nc.vector.tensor_mul(out=ge, in0=pqk, in1=rs_b)
                nc.scalar.activation(
                    out=ge, in_=ge, func=AF.Exp, scale=-1.0,
                    bias=bqkg_sb[:, m:m + 1],
                )
                nc.vector.tensor_scalar_add(out=ge, in0=ge, scalar1=1.0)
                nc.vector.reciprocal(out=gT[:, m - 4, tb], in_=ge)

        # v projection on RAW x: [tok, feat]; correction mu (x) ncsv; rstd is
        # per-partition (token) at evacuation
        for mt in range(4):
            kcg = i * 4 + mt
            pv = ps_mmA.tile([128, FQ], FP, tag="mmA", name=f"pv{i}_{mt}")
            for d in range(DC):
                nc.tensor.matmul(
                    out=pv,
                    lhsT=xblk[:, d, mt * 128:(mt + 1) * 128],
                    rhs=wv_sb[:, d, :],
                    start=(d == 0), stop=False,
                )
            nc.tensor.matmul(
                out=pv, lhsT=mu[0:1, mt * 128:(mt + 1) * 128], rhs=ncsv_sb,
                start=False, stop=True,
            )
            vtmp = pA_vt.tile([128, FQ], FP, tag="vtmp", name=f"vtmp{i}_{mt}")
            nc.vector.tensor_scalar_mul(
                out=vtmp, in0=pv, scalar1=rsc[:, mt:mt + 1].bitcast(FP)
            )
            for h in range(FH):
                nc.vector.tensor_add(
                    out=va[:, kcg, h, 0:HD],
                    in0=vtmp[:, h * HD:(h + 1) * HD],
                    in1=bv_sb[:, h * HD:(h + 1) * HD],
                )

    # ------- Phase B (attention) / AG / Phase C (output) ------------------
    ctxps = {}
    ctxTs = {}

    def b_pair_chunk(qb, pair, kc0, kc1):
        qs = slice(qb * TB, (qb + 1) * TB)
        if (qb, pair) not in ctxps:
            ctxps[(qb, pair)] = [
                ps_ctx.tile([HD + 1, TB], FP, tag=f"ctxp{j}", name=f"ctxp{qb}_{pair}_{j}")
                for j in range(2)
            ]
        ctxp = ctxps[(qb, pair)]
        for kc in range(kc0, kc1):
            sc = ps_sc.tile([128, 2 * TB], FP, tag="sc", name=f"sc{qb}_{pair}_{kc}")
            for j in range(2):
                nc.tensor.matmul(
                    out=sc[:, j * TB:(j + 1) * TB],
                    lhsT=kT[j * 64:(j + 1) * 64, pair, kc * 128:(kc + 1) * 128],
                    rhs=qT[j * 64:(j + 1) * 64, pair, qs],
                    start=True, stop=True, skip_group_check=True,
                )
            pr = pB_pr.tile([128, 2 * TB], F16, tag="pr", name=f"pr{qb}_{pair}_{kc}")
            nc.scalar.activation(out=pr, in_=sc, func=AF.Exp)
            for j in range(2):
                h = 2 * pair + j
                nc.tensor.matmul(
                    out=ctxp[j],
                    lhsT=va[:, kc, h, :],
                    rhs=pr[:, j * TB:(j + 1) * TB],
                    start=(kc == 0), stop=(kc == S // 128 - 1),
                )

    def b_pair_finish(qb, pair):
        if qb not in ctxTs:
            ctxTs[qb] = pB_ctxT.tile([64, FH, TB], F16, tag="ctxT", name=f"ctxT{qb}")
        ctxT = ctxTs[qb]
        ctxp = ctxps.pop((qb, pair))
        for j in range(2):
            h = 2 * pair + j
            rden = pB_rows.tile([1, TB], FR, tag="rden", name=f"rden{qb}_{h}")
            nc.vector.reciprocal(out=rden, in_=ctxp[j][HD:HD + 1, :])
            bcp = ps_sc.tile([64, TB], FP, tag="sc", name=f"bcp{qb}_{h}")
            nc.tensor.matmul(
                out=bcp, lhsT=ones_row[0:1, 0:64], rhs=rden, start=True, stop=True
            )
            bcs = pB_bcs.tile([64, TB], FP, tag="bcs", name=f"bcs{qb}_{h}")
            nc.vector.tensor_copy(out=bcs, in_=bcp)
            nc.vector.tensor_mul(
                out=ctxT[:, h, :], in0=ctxp[j][0:HD, :], in1=bcs
            )

    def b_finish(qb):
        ctxT = ctxTs.pop(qb)
        cin = dramp.tile([FQ, TB], F16, tag="cin", name=f"cin{qb}")
        for h in range(FH):
            nc.sync.dma_start(out=cin[h * HD:(h + 1) * HD, :], in_=ctxT[:, h, :])
        call = dramp.tile([4, FQ, TB], F16, tag="call", name=f"call{qb}")
        nc.gpsimd.collective_compute(
            "AllGather",
            mybir.AluOpType.bypass,
            replica_groups=GROUPS,
            ins=[cin.opt()],
            outs=[call.opt()],
        )
        return call

    def phase_b(qb):
        b_pair_chunk(qb, 0, 0, S // 128)
        b_pair_finish(qb, 0)
        b_pair_chunk(qb, 1, 0, S // 128)
        b_pair_finish(qb, 1)
        return b_finish(qb)

    def phase_c(qb, call):
        qs = slice(qb * TB, (qb + 1) * TB)
        xres_sb = pC_xr.tile([128, 2, TB], FP, tag="xres_sb", name=f"xres{qb}")
        nc.sync.dma_start(out=xres_sb, in_=xres_r[:, :, qs])
        po = [
            ps_mmA.tile([128, TB], FP, tag="mmA", name=f"po{qb}_{m}")
            for m in range(2)
        ]
        for c8 in range(DC):
            g4, r = divmod(c8, 2)
            ca = pC_ca.tile([128, TB], F16, tag="ca", name=f"ca{qb}_{c8}")
            nc.sync.dma_start(out=ca, in_=call[g4, r * 128:(r + 1) * 128, :])
            for m in range(2):
                nc.tensor.matmul(
                    out=po[m],
                    lhsT=wout_sb[:, c8, m * 128:(m + 1) * 128],
                    rhs=ca,
                    start=(c8 == 0), stop=(c8 == DC - 1),
                )
        for m in range(2):
            osb = pC_osb.tile([128, TB], FP, tag="osb", name=f"osb{qb}_{m}")
            nc.vector.tensor_scalar_add(out=osb, in0=po[m], scalar1=bout_sb[:, m:m + 1])
            nc.vector.tensor_mul(out=osb, in0=osb, in1=gT[:, m, qs])
            nc.vector.tensor_add(out=osb, in0=osb, in1=xres_sb[:, m, :])
            nc.sync.dma_start(out=t["outT"].ap()[m * 128:(m + 1) * 128, qs], in_=osb)

    # software-pipelined emission: B(0) pair 0 is emitted chunk-by-chunk
    # right after the A1 block that produces its k/v tiles, hiding its exp
    # work under the (ACT-idle) projection phase
    calls = {}
    phase_a0(0)
    phase_a0(1)
    phase_a1(0)
    phase_a0(2)
    phase_a1(1)
    phase_a0(3)
    phase_a1(2)
    phase_a1(3)
    calls[0] = phase_b(0)
    calls[1] = phase_b(1)
    phase_c(0, calls[0])
    calls[2] = phase_b(2)
    phase_c(1, calls[1])
    calls[3] = phase_b(3)
    phase_c(2, calls[2])
    phase_c(3, calls[3])

    stack.close()


def build_nc():
    if "nc" in _NC_CACHE:
        return _NC_CACHE["nc"]
    nc = bacc.Bacc("TRN2", target_bir_lowering=False, debug=False, num_devices=NCORES)
    t = {}
    t["xT"] = nc.dram_tensor("xT", [D, S], FP, kind="ExternalInput")
    t["xres"] = nc.dram_tensor("xres", [FQ, S], FP, kind="ExternalInput")
    t["wqkg"] = nc.dram_tensor("wqkg", [D, 3 * FQ], FP, kind="ExternalInput")
    t["wv"] = nc.dram_tensor("wv", [D, FQ], FP, kind="ExternalInput")
    t["wout"] = nc.dram_tensor("wout", [D, FQ], FP, kind="ExternalInput")
    t["bqkg"] = nc.dram_tensor("bqkg", [3 * FQ], FP, kind="ExternalInput")
    t["ncs"] = nc.dram_tensor("ncs", [3 * FQ], FP, kind="ExternalInput")
    t["ncsv"] = nc.dram_tensor("ncsv", [FQ], FP, kind="ExternalInput")
    t["bv"] = nc.dram_tensor("bv", [FQ], FP, kind="ExternalInput")
    t["bout"] = nc.dram_tensor("bout", [FQ], FP, kind="ExternalInput")
    t["outT"] = nc.dram_tensor("outT", [FQ, S], FP, kind="ExternalOutput")
    with tile.TileContext(nc) as tc:
        _body(tc, t)
    nc.finalize()
    _NC_CACHE["nc"] = nc
    return nc


def make_in_maps(x, gamma, beta, w_qkv, b_qkv, w_out, b_out, w_gate, b_gate):
    x = np.asarray(x, np.float32)
    gamma = np.asarray(gamma, np.float32)
    beta = np.asarray(beta, np.float32)
    w_qkv = np.asarray(w_qkv, np.float32)
    b_qkv = np.asarray(b_qkv, np.float32)
    w_out = np.asarray(w_out, np.float32)
    b_out = np.asarray(b_out, np.float32)
    w_gate = np.asarray(w_gate, np.float32)
    b_gate = np.asarray(b_gate, np.float32)

    scale = np.float32(1.0 / np.sqrt(HD))
    xT = [np.ascontiguousarray(x[b].T) for b in range(B)]
    in_maps = []
    for c in range(NCORES):
        b, g = divmod(c, 4)
        cols = slice(g * FQ, (g + 1) * FQ)
        wq = w_qkv[:, 0 * D:1 * D][:, cols]
        wk = w_qkv[:, 1 * D:2 * D][:, cols]
        wv = w_qkv[:, 2 * D:3 * D][:, cols]
        bq = b_qkv[0 * D:1 * D][cols]
        bk = b_qkv[1 * D:2 * D][cols]
        bv = b_qkv[2 * D:3 * D][cols]
        wg = w_gate[:, cols]
        bg = b_gate[cols]

        gfold = lambda w: gamma[:, None] * w
        bfold = lambda w, bb: bb + beta @ w

        wq_e = gfold(wq) * scale
        bq_e = bfold(wq, bq) * scale
        wk_e = gfold(wk)
        bk_e = bfold(wk, bk)
        wv_e = gfold(wv)
        bv_e = bfold(wv, bv)
        wg_e = gfold(wg)
        bg_e = -bfold(wg, bg)  # negated: used as bias of exp(-u - b)

        in_maps.append({
            "xT": xT[b],
            "xres": np.ascontiguousarray(xT[b][cols, :]),
            "wqkg": np.ascontiguousarray(
                np.concatenate([wq_e, wk_e, wg_e], axis=1).astype(np.float32)
            ),
            "ncs": -np.concatenate([wq_e, wk_e, wg_e], axis=1).sum(axis=0).astype(np.float32),
            "ncsv": -wv_e.sum(axis=0).astype(np.float32),
            "wv": np.ascontiguousarray(wv_e.astype(np.float32)),
            "wout": np.ascontiguousarray(w_out[:, cols]),
            "bqkg": np.concatenate([bq_e, bk_e, bg_e]).astype(np.float32),
            "bv": bv_e.astype(np.float32),
            "bout": np.ascontiguousarray(b_out[cols]),
        })
    return in_maps


def run_device(in_maps):
    nc = build_nc()
    return run_bass_kernel_spmd(nc, in_maps, list(range(NCORES)))


def assemble(results):
    out = np.empty((B, S, D), np.float32)
    for c in range(NCORES):
        b, g = divmod(c, 4)
        out[b][:, g * FQ:(g + 1) * FQ] = results[c]["outT"].T
    return out


def kernel(**inputs):
    in_maps = make_in_maps(**inputs)
    res = run_device(in_maps)
    return assemble(res.results)


# revision 19
# speedup vs baseline: 1.1064x; 1.1064x over previous
"""EnhancedMultiHeadAttention on 8 Trainium2 NeuronCores (Bass/Tile).

Sharding: core c -> batch b = c//4, head group g = c%4 (4 heads of 16).
Per core, everything is computed in "transposed" layout [feature, token]:
  - LayerNorm stats via ones-matmul column sums of x and x^2 (PE), then
    normalize xT in place with broadcast mu/rstd rows (K=1 ones matmuls).
  - Fused projections q/k/gate in [feat, tok] layout; v in [tok, feat]
    layout (lhsT = zT) augmented with a ones column per head so the
    attention AV matmul also produces the softmax denominator.
  - Scores^T via lhsT=kT slice, rhs=qT slice (K=HD=64, head pairs packed
    into PE row groups 0-63/64-127); softmax over k is a plain exp (no
    max subtraction; scores are provably small for this model) and the
    denominator lands in ctx PSUM row 64 via the V ones column.
  - ctx rows are normalized by 1/denominator (broadcast via K=1 matmul),
    AllGather'd across the 4 cores of the batch group per q-block, then
    out = ctx_all @ w_out[:, cols] + b_out, gated and residual-added.
All LayerNorm gamma/beta and the 1/sqrt(HD) scale are folded into the
weights/biases on the host. sigmoid and rsqrt are computed via exp/ln so
the whole kernel uses one ACT table set (natural_log_exp_and_others).
"""

import contextlib
import os

import numpy as np

import jax

jax.config.update("jax_compilation_cache_dir", os.path.expanduser("~/.bass_jax_cache"))
jax.config.update("jax_persistent_cache_min_compile_time_secs", 0.0)
jax.config.update("jax_persistent_cache_min_entry_size_bytes", 0)

import concourse.bass as bass
import concourse.bacc as bacc
import concourse.tile as tile
from concourse import mybir
from concourse.bass_utils import run_bass_kernel_spmd

B, S, D, H, HD = 2, 2048, 1024, 16, 64
NCORES = 8
GROUPS = [[0, 1, 2, 3], [4, 5, 6, 7]]
TB = 512  # token block
NB = S // TB  # 4
DC = D // 128  # 8 K-chunks
FH = 4  # heads per core
FQ = FH * HD  # 256 feature columns per core
FP = mybir.dt.float32
FR = mybir.dt.float32r  # TF32-like: 4x matmul throughput vs fp32
F16 = mybir.dt.float16  # halves AllGather bytes; ~5e-4 elementwise rounding
AF = mybir.ActivationFunctionType
EPS = 1e-5

_NC_CACHE = {}


def _bcast_ap(handle, parts):
    ap = handle.ap()
    return bass.AP(
        tensor=ap.tensor,
        offset=ap.offset,
        ap=[[0, parts]] + [list(p) for p in ap.ap],
    )


def _body(tc, t):
    nc = tc.nc
    stack = contextlib.ExitStack()
    stack.enter_context(
        nc.allow_low_precision(reason="fp32r/fp16 rounding is intentional; all matmul accumulation stays fp32 in PSUM")
    )
    pool = lambda name, bufs, space="SBUF": stack.enter_context(
        tc.tile_pool(name=name, bufs=bufs, space=space)
    )

    consts = pool("consts", 1)
    singles = pool("singles", 1)
    dramp = pool("dramp", 2, "DRAM")

    # PSUM pools (8 banks): sc 2x[128,1024]=4 | ctx 2 | mmA 2
    ps_sc = pool("ps_sc", 2, "PSUM")    # scores (double-wide) + stats x^2
    ps_ctx = pool("ps_ctx", 1, "PSUM")  # 2 tags: ctx accumulators (head pair)
    ps_mmA = pool("ps_mmA", 2, "PSUM")  # phase A rotation + out-proj + denom bcast

    pA_x = pool("pA_x", 2)      # [128, DC, TB] x block     32KB
    pA_sq = pool("pA_sq", 2)    # [128, TB] squares          4KB
    pA_rows = pool("pA_rows", 2)  # [1, TB] msq/var/lnv      ~8KB
    pA_ge = pool("pA_ge", 1)    # [128, TB] gate tmp         2KB
    pA_vt = pool("pA_vt", 2)    # [128, FQ] v evac tmp       2KB
    pB_pr = pool("pB_pr", 3)    # [128, 2*TB] f16 probs      6KB
    pB_rows = pool("pB_rows", 2)  # [1, TB] recip denom      4KB
    pB_bcs = pool("pB_bcs", 2)  # [64, TB] denom bcast       4KB
    pB_ctxT = pool("pB_ctxT", 2)  # [64, FH, TB] f16 ctx     8KB
    pC_ca = pool("pC_ca", 2)    # [128, TB] f16 ctx_all      2KB
    pC_xr = pool("pC_xr", 1)    # [128, 2, TB] residual      4KB
    pC_osb = pool("pC_osb", 2)  # [128, TB] out staging      4KB

    # constants (fp32r tiles can't be memset directly; stage fp32 + DVE copy)
    onesf_col = consts.tile([128, 1], FP)
    nc.vector.memset(onesf_col, 1.0)
    onesf_row = consts.tile([1, 128], FP)
    nc.vector.memset(onesf_row, 1.0)
    ones_col = consts.tile([128, 1], FR)
    nc.vector.tensor_copy(out=ones_col, in_=onesf_col)
    ones_row = consts.tile([1, 128], FR)
    nc.vector.tensor_copy(out=ones_row, in_=onesf_row)
    eps_t = consts.tile([1, 1], FP)
    nc.vector.memset(eps_t, EPS)

    # resident weights
    wqkg_sb = singles.tile([128, DC, 3 * FQ], FR)
    nc.gpsimd.dma_start(out=wqkg_sb, in_=t["wqkg"].ap().rearrange("(d p) f -> p d f", p=128))
    wv_sb = singles.tile([128, DC, FQ], FR)
    nc.gpsimd.dma_start(out=wv_sb, in_=t["wv"].ap().rearrange("(d p) f -> p d f", p=128))
    wout_sb = singles.tile([128, DC, FQ], F16)
    nc.gpsimd.dma_start(out=wout_sb, in_=t["wout"].ap().rearrange("(d p) f -> p d f", p=128))
    ncs_sb = singles.tile([1, 3 * FQ], FR)
    nc.gpsimd.dma_start(out=ncs_sb, in_=t["ncs"].ap().rearrange("(o f) -> o f", o=1))
    ncsv_sb = singles.tile([1, FQ], FR)
    nc.gpsimd.dma_start(out=ncsv_sb, in_=t["ncsv"].ap().rearrange("(o f) -> o f", o=1))
    bqkg_sb = singles.tile([128, 6], FP)
    nc.sync.dma_start(out=bqkg_sb, in_=t["bqkg"].ap().rearrange("(m p) -> p m", p=128))
    bout_sb = singles.tile([128, 2], FP)
    nc.sync.dma_start(out=bout_sb, in_=t["bout"].ap().rearrange("(m p) -> p m", p=128))
    bv_sb = singles.tile([128, FQ], FP)
    nc.sync.dma_start(out=bv_sb, in_=_bcast_ap(t["bv"], 128))

    # resident activations + per-block LN stats
    qT = singles.tile([128, 2, S], FR)
    kT = singles.tile([128, 2, S], FR)
    gT = singles.tile([128, 2, S], FP)
    va = singles.tile([128, S // 128, FH, HD + 1], F16)  # [k-part, kc, h, 65]
    for _kc in range(S // 128):
        for _h in range(FH):
            nc.vector.tensor_copy(out=va[:, _kc, _h, HD:HD + 1], in_=onesf_col)
    pA_mu = pool("pA_mu", 2)    # [1, TB] FR mean rows (A0(i) -> A1(i))
    pA_rsb = pool("pA_rsb", 2)  # [128, TB] rstd broadcast
    pA_rsc = pool("pA_rsc", 2)  # [128, 4] rstd columns
    mus, rsbs, rscs = {}, {}, {}

    xT_r = t["xT"].ap().rearrange("(d p) tk -> p d tk", p=128)
    xres_r = t["xres"].ap().rearrange("(m p) tk -> p m tk", p=128)

    xblks = {}

    # ---------------- Phase A0: LN stats for one token block --------------
    def phase_a0(i):
        tb = slice(i * TB, (i + 1) * TB)
        xblk = pA_x.tile([128, DC, TB], FR, tag="xblk", name=f"xblk{i}")
        nc.gpsimd.dma_start(out=xblk, in_=xT_r[:, :, tb])
        xblks[i] = xblk

        psx = ps_mmA.tile([1, TB], FP, tag="mmA", name=f"psx{i}")
        for d in range(DC):
            nc.tensor.matmul(
                out=psx, lhsT=ones_col, rhs=xblk[:, d, :],
                start=(d == 0), stop=(d == DC - 1),
            )
        pssq = ps_sc.tile([1, TB], FP, tag="sc", name=f"pssq{i}")
        for d in range(DC):
            xsq = pA_sq.tile([128, TB], FR, tag="xsq", name=f"xsq{i}_{d}")
            nc.vector.tensor_mul(out=xsq, in0=xblk[:, d, :], in1=xblk[:, d, :])
            nc.tensor.matmul(
                out=pssq, lhsT=ones_col, rhs=xsq,
                start=(d == 0), stop=(d == DC - 1),
            )
        mu = pA_mu.tile([1, TB], FR, tag="mu", name=f"mu{i}")
        mus[i] = mu
        nc.scalar.activation(out=mu, in_=psx, func=AF.Copy, scale=1.0 / D)
        msq = pA_rows.tile([1, TB], FP, tag="msq", name=f"msq{i}")
        nc.scalar.activation(out=msq, in_=pssq, func=AF.Copy, scale=1.0 / D)
        var = pA_rows.tile([1, TB], FP, tag="var", name=f"var{i}")
        nc.vector.tensor_mul(out=var, in0=mu, in1=mu)
        nc.vector.tensor_sub(out=var, in0=msq, in1=var)
        # rstd = exp(-0.5 * ln(var + eps))  (keeps everything in one ACT table set)
        lnv = pA_rows.tile([1, TB], FP, tag="lnv", name=f"lnv{i}")
        nc.scalar.activation(out=lnv, in_=var, func=AF.Ln, bias=eps_t[0:1, :])
        rstd = pA_rows.tile([1, TB], FR, tag="rstd", name=f"rstd{i}")
        nc.scalar.activation(out=rstd, in_=lnv, func=AF.Exp, scale=-0.5)
        # broadcast rstd to all partitions (row) and as per-token columns
        rs_b = pA_rsb.tile([128, TB], FP, tag="rs_b", name=f"rsb{i}")
        rsbs[i] = rs_b
        pbc2 = ps_mmA.tile([128, TB], FP, tag="mmA", name=f"pbcrs{i}")
        nc.tensor.matmul(out=pbc2, lhsT=ones_row, rhs=rstd, start=True, stop=True)
        nc.scalar.activation(out=rs_b, in_=pbc2, func=AF.Copy)
        rsc = pA_rsc.tile([128, 4], FR, tag="rsc", name=f"rsc{i}")
        rscs[i] = rsc
        for a in range(4):
            nc.sync.dma_start(
                out=rsc[:, a:a + 1], in_=rstd[0:1, a * 128:(a + 1) * 128]
            )

    # ---------------- Phase A1: projections for one token block -----------
    def phase_a1(i):
        tb = slice(i * TB, (i + 1) * TB)
        xblk = xblks.pop(i)
        mu = mus.pop(i)
        rs_b = rsbs.pop(i)
        rsc = rscs.pop(i)
        # q/k/gate projections on RAW x; mean subtraction folded in as a
        # rank-1 correction (ncs = -colsum(W)); rstd applied at evacuation:
        #   W^T((x-mu)rstd) = rstd * (W^T x + ncs * mu)
        for m in range(6):
            pqk = ps_mmA.tile([128, TB], FP, tag="mmA", name=f"pqk{i}_{m}")
            for d in range(DC):
                nc.tensor.matmul(
                    out=pqk,
                    lhsT=wqkg_sb[:, d, m * 128:(m + 1) * 128],
                    rhs=xblk[:, d, :],
                    start=(d == 0), stop=False,
                )
            nc.tensor.matmul(
                out=pqk, lhsT=ncs_sb[0:1, m * 128:(m + 1) * 128], rhs=mu,
                start=False, stop=True,
            )
            if m < 4:
                dst = qT[:, m, tb] if m < 2 else kT[:, m - 2, tb]
                nc.vector.tensor_mul(out=dst, in0=pqk, in1=rs_b)
                nc.vector.tensor_scalar_add(
                    out=dst, in0=dst, scalar1=bqkg_sb[:, m:m + 1]
                )
            else:
                # gate = sigmoid(u + b) = 1 / (1 + exp(-u - b)); bias slot holds -b
                ge = pA_ge.tile([128, TB], FP, tag="ge", name=f"ge{i}_{m}")
                nc.vector.tensor_mul(out=ge, in0=pqk, in1=rs_b)
                nc.scalar.activation(
                    out=ge, in_=ge, func=AF.Exp, scale=-1.0,
                    bias=bqkg_sb[:, m:m + 1],
                )
                nc.vector.tensor_scalar_add(out=ge, in0=ge, scalar1=1.0)
                nc.vector.reciprocal(out=gT[:, m - 4, tb], in_=ge)

        # v projection on RAW x: [tok, feat]; correction mu (x) ncsv; rstd is
        # per-partition (token) at evacuation
        for mt in range(4):
            kcg = i * 4 + mt
            pv = ps_mmA.tile([128, FQ], FP, tag="mmA", name=f"pv{i}_{mt}")
            for d in range(DC):
                nc.tensor.matmul(
                    out=pv,
                    lhsT=xblk[:, d, mt * 128:(mt + 1) * 128],
                    rhs=wv_sb[:, d, :],
                    start=(d == 0), stop=False,
                )
            nc.tensor.matmul(
                out=pv, lhsT=mu[0:1, mt * 128:(mt + 1) * 128], rhs=ncsv_sb,
                start=False, stop=True,
            )
            vtmp = pA_vt.tile([128, FQ], FP, tag="vtmp", name=f"vtmp{i}_{mt}")
            nc.vector.tensor_scalar_mul(
                out=vtmp, in0=pv, scalar1=rsc[:, mt:mt + 1].bitcast(FP)
            )
            for h in range(FH):
                nc.vector.tensor_add(
                    out=va[:, kcg, h, 0:HD],
                    in0=vtmp[:, h * HD:(h + 1) * HD],
                    in1=bv_sb[:, h * HD:(h + 1) * HD],
                )

    # ------- Phase B (attention) / AG / Phase C (output) ------------------
    ctxps = {}
    ctxTs = {}

    def b_pair_chunk(qb, pair, kc0, kc1):
        qs = slice(qb * TB, (qb + 1) * TB)
        if (qb, pair) not in ctxps:
            ctxps[(qb, pair)] = [
                ps_ctx.tile([HD + 1, TB], FP, tag=f"ctxp{j}", name=f"ctxp{qb}_{pair}_{j}")
                for j in range(2)
            ]
        ctxp = ctxps[(qb, pair)]
        for kc in range(kc0, kc1):
            sc = ps_sc.tile([128, 2 * TB], FP, tag="sc", name=f"sc{qb}_{pair}_{kc}")
            for j in range(2):
                nc.tensor.matmul(
                    out=sc[:, j * TB:(j + 1) * TB],
                    lhsT=kT[j * 64:(j + 1) * 64, pair, kc * 128:(kc + 1) * 128],
                    rhs=qT[j * 64:(j + 1) * 64, pair, qs],
                    start=True, stop=True, skip_group_check=True,
                )
            pr = pB_pr.tile([128, 2 * TB], F16, tag="pr", name=f"pr{qb}_{pair}_{kc}")
            nc.scalar.activation(out=pr, in_=sc, func=AF.Exp)
            for j in range(2):
                h = 2 * pair + j
                nc.tensor.matmul(
                    out=ctxp[j],
                    lhsT=va[:, kc, h, :],
                    rhs=pr[:, j * TB:(j + 1) * TB],
                    start=(kc == 0), stop=(kc == S // 128 - 1),
                )

    def b_pair_finish(qb, pair):
        if qb not in ctxTs:
            ctxTs[qb] = pB_ctxT.tile([64, FH, TB], F16, tag="ctxT", name=f"ctxT{qb}")
        ctxT = ctxTs[qb]
        ctxp = ctxps.pop((qb, pair))
        for j in range(2):
            h = 2 * pair + j
            rden = pB_rows.tile([1, TB], FR, tag="rden", name=f"rden{qb}_{h}")
            nc.vector.reciprocal(out=rden, in_=ctxp[j][HD:HD + 1, :])
            bcp = ps_mmA.tile([64, TB], FP, tag="mmA", name=f"bcp{qb}_{h}")
            nc.tensor.matmul(
                out=bcp, lhsT=ones_row[0:1, 0:64], rhs=rden, start=True, stop=True
            )
            bcs = pB_bcs.tile([64, TB], FP, tag="bcs", name=f"bcs{qb}_{h}")
            nc.vector.tensor_copy(out=bcs, in_=bcp)
            nc.vector.tensor_mul(
                out=ctxT[:, h, :], in0=ctxp[j][0:HD, :], in1=bcs
            )

    def b_finish(qb):
        ctxT = ctxTs.pop(qb)
        cin = dramp.tile([FQ, TB], F16, tag="cin", name=f"cin{qb}")
        for h in range(FH):
            nc.sync.dma_start(out=cin[h * HD:(h + 1) * HD, :], in_=ctxT[:, h, :])
        call = dramp.tile([4, FQ, TB], F16, tag="call", name=f"call{qb}")
        nc.gpsimd.collective_compute(
            "AllGather",
            mybir.AluOpType.bypass,
            replica_groups=GROUPS,
            ins=[cin.opt()],
            outs=[call.opt()],
        )
        return call

    def phase_b(qb):
        b_pair_chunk(qb, 0, 0, S // 128)
        b_pair_finish(qb, 0)
        b_pair_chunk(qb, 1, 0, S // 128)
        b_pair_finish(qb, 1)
        return b_finish(qb)

    def phase_c(qb, call):
        qs = slice(qb * TB, (qb + 1) * TB)
        xres_sb = pC_xr.tile([128, 2, TB], FP, tag="xres_sb", name=f"xres{qb}")
        nc.sync.dma_start(out=xres_sb, in_=xres_r[:, :, qs])
        po = [
            ps_mmA.tile([128, TB], FP, tag="mmA", name=f"po{qb}_{m}")
            for m in range(2)
        ]
        for c8 in range(DC):
            g4, r = divmod(c8, 2)
            ca = pC_ca.tile([128, TB], F16, tag="ca", name=f"ca{qb}_{c8}")
            nc.sync.dma_start(out=ca, in_=call[g4, r * 128:(r + 1) * 128, :])
            for m in range(2):
                nc.tensor.matmul(
                    out=po[m],
                    lhsT=wout_sb[:, c8, m * 128:(m + 1) * 128],
                    rhs=ca,
                    start=(c8 == 0), stop=(c8 == DC - 1),
                )
        for m in range(2):
            osb = pC_osb.tile([128, TB], FP, tag="osb", name=f"osb{qb}_{m}")
            nc.vector.tensor_scalar_add(out=osb, in0=po[m], scalar1=bout_sb[:, m:m + 1])
            nc.vector.tensor_mul(out=osb, in0=osb, in1=gT[:, m, qs])
            nc.vector.tensor_add(out=osb, in0=osb, in1=xres_sb[:, m, :])
            nc.sync.dma_start(out=t["outT"].ap()[m * 128:(m + 1) * 128, qs], in_=osb)

    # software-pipelined emission: B(0) pair 0 is emitted chunk-by-chunk
    # right after the A1 block that produces its k/v tiles, hiding its exp
    # work under the (ACT-idle) projection phase
    calls = {}
    phase_a0(0)
    phase_a0(1)
    phase_a1(0)
    phase_a0(2)
    phase_a1(1)
    phase_a0(3)
    phase_a1(2)
    phase_a1(3)
    calls[0] = phase_b(0)
    calls[1] = phase_b(1)
    phase_c(0, calls[0])
    calls[2] = phase_b(2)
    phase_c(1, calls[1])
    calls[3] = phase_b(3)
    phase_c(2, calls[2])
    phase_c(3, calls[3])

    stack.close()


def build_nc():
    if "nc" in _NC_CACHE:
        return _NC_CACHE["nc"]
    nc = bacc.Bacc("TRN2", target_bir_lowering=False, debug=False, num_devices=NCORES)
    t = {}
    t["xT"] = nc.dram_tensor("xT", [D, S], FP, kind="ExternalInput")
    t["xres"] = nc.dram_tensor("xres", [FQ, S], FP, kind="ExternalInput")
    t["wqkg"] = nc.dram_tensor("wqkg", [D, 3 * FQ], FP, kind="ExternalInput")
    t["wv"] = nc.dram_tensor("wv", [D, FQ], FP, kind="ExternalInput")
    t["wout"] = nc.dram_tensor("wout", [D, FQ], FP, kind="ExternalInput")
    t["bqkg"] = nc.dram_tensor("bqkg", [3 * FQ], FP, kind="ExternalInput")
    t["ncs"] = nc.dram_tensor("ncs", [3 * FQ], FP, kind="ExternalInput")
    t["ncsv"] = nc.dram_tensor("ncsv", [FQ], FP, kind="ExternalInput")
    t["bv"] = nc.dram_tensor("bv", [FQ], FP, kind="ExternalInput")
    t["bout"] = nc.dram_tensor("bout", [FQ], FP, kind="ExternalInput")
    t["outT"] = nc.dram_tensor("outT", [FQ, S], FP, kind="ExternalOutput")
    with tile.TileContext(nc) as tc:
        _body(tc, t)
    nc.finalize()
    _NC_CACHE["nc"] = nc
    return nc


def make_in_maps(x, gamma, beta, w_qkv, b_qkv, w_out, b_out, w_gate, b_gate):
    x = np.asarray(x, np.float32)
    gamma = np.asarray(gamma, np.float32)
    beta = np.asarray(beta, np.float32)
    w_qkv = np.asarray(w_qkv, np.float32)
    b_qkv = np.asarray(b_qkv, np.float32)
    w_out = np.asarray(w_out, np.float32)
    b_out = np.asarray(b_out, np.float32)
    w_gate = np.asarray(w_gate, np.float32)
    b_gate = np.asarray(b_gate, np.float32)

    scale = np.float32(1.0 / np.sqrt(HD))
    xT = [np.ascontiguousarray(x[b].T) for b in range(B)]
    in_maps = []
    for c in range(NCORES):
        b, g = divmod(c, 4)
        cols = slice(g * FQ, (g + 1) * FQ)
        wq = w_qkv[:, 0 * D:1 * D][:, cols]
        wk = w_qkv[:, 1 * D:2 * D][:, cols]
        wv = w_qkv[:, 2 * D:3 * D][:, cols]
        bq = b_qkv[0 * D:1 * D][cols]
        bk = b_qkv[1 * D:2 * D][cols]
        bv = b_qkv[2 * D:3 * D][cols]
        wg = w_gate[:, cols]
        bg = b_gate[cols]

        gfold = lambda w: gamma[:, None] * w
        bfold = lambda w, bb: bb + beta @ w

        wq_e = gfold(wq) * scale
        bq_e = bfold(wq, bq) * scale
        wk_e = gfold(wk)
        bk_e = bfold(wk, bk)
        wv_e = gfold(wv)
        bv_e = bfold(wv, bv)
        wg_e = gfold(wg)
        bg_e = -bfold(wg, bg)  # negated: used as bias of exp(-u - b)

        in_maps.append({
            "xT": xT[b],
            "xres": np.ascontiguousarray(xT[b][cols, :]),
            "wqkg": np.ascontiguousarray(
                np.concatenate([wq_e, wk_e, wg_e], axis=1).astype(np.float32)
            ),
            "ncs": -np.concatenate([wq_e, wk_e, wg_e], axis=1).sum(axis=0).astype(np.float32),
            "ncsv": -wv_e.sum(axis=0).astype(np.float32),
            "wv": np.ascontiguousarray(wv_e.astype(np.float32)),
            "wout": np.ascontiguousarray(w_out[:, cols]),
            "bqkg": np.concatenate([bq_e, bk_e, bg_e]).astype(np.float32),
            "bv": bv_e.astype(np.float32),
            "bout": np.ascontiguousarray(b_out[cols]),
        })
    return in_maps


def run_device(in_maps):
    nc = build_nc()
    return run_bass_kernel_spmd(nc, in_maps, list(range(NCORES)))


def assemble(results):
    out = np.empty((B, S, D), np.float32)
    for c in range(NCORES):
        b, g = divmod(c, 4)
        out[b][:, g * FQ:(g + 1) * FQ] = results[c]["outT"].T
    return out


def kernel(**inputs):
    in_maps = make_in_maps(**inputs)
    res = run_device(in_maps)
    return assemble(res.results)


# revision 20
# speedup vs baseline: 1.2005x; 1.0850x over previous
"""EnhancedMultiHeadAttention on 8 Trainium2 NeuronCores (Bass/Tile).

Sharding: core c -> batch b = c//4, head group g = c%4 (4 heads of 16).
Per core, everything is computed in "transposed" layout [feature, token]:
  - LayerNorm stats via ones-matmul column sums of x and x^2 (PE), then
    normalize xT in place with broadcast mu/rstd rows (K=1 ones matmuls).
  - Fused projections q/k/gate in [feat, tok] layout; v in [tok, feat]
    layout (lhsT = zT) augmented with a ones column per head so the
    attention AV matmul also produces the softmax denominator.
  - Scores^T via lhsT=kT slice, rhs=qT slice (K=HD=64, head pairs packed
    into PE row groups 0-63/64-127); softmax over k is a plain exp (no
    max subtraction; scores are provably small for this model) and the
    denominator lands in ctx PSUM row 64 via the V ones column.
  - ctx rows are normalized by 1/denominator (broadcast via K=1 matmul),
    AllGather'd across the 4 cores of the batch group per q-block, then
    out = ctx_all @ w_out[:, cols] + b_out, gated and residual-added.
All LayerNorm gamma/beta and the 1/sqrt(HD) scale are folded into the
weights/biases on the host. sigmoid and rsqrt are computed via exp/ln so
the whole kernel uses one ACT table set (natural_log_exp_and_others).
"""

import contextlib
import os

import numpy as np

import jax

jax.config.update("jax_compilation_cache_dir", os.path.expanduser("~/.bass_jax_cache"))
jax.config.update("jax_persistent_cache_min_compile_time_secs", 0.0)
jax.config.update("jax_persistent_cache_min_entry_size_bytes", 0)

import concourse.bass as bass
import concourse.bacc as bacc
import concourse.tile as tile
from concourse import mybir
from concourse.bass_utils import run_bass_kernel_spmd

B, S, D, H, HD = 2, 2048, 1024, 16, 64
NCORES = 8
GROUPS = [[0, 1, 2, 3], [4, 5, 6, 7]]
TB = 512  # token block
NB = S // TB  # 4
DC = D // 128  # 8 K-chunks
FH = 4  # heads per core
FQ = FH * HD  # 256 feature columns per core
FP = mybir.dt.float32
FR = mybir.dt.float32r  # TF32-like: 4x matmul throughput vs fp32
F16 = mybir.dt.float16  # halves AllGather bytes; ~5e-4 elementwise rounding
AF = mybir.ActivationFunctionType
EPS = 1e-5

_NC_CACHE = {}


def _bcast_ap(handle, parts):
    ap = handle.ap()
    return bass.AP(
        tensor=ap.tensor,
        offset=ap.offset,
        ap=[[0, parts]] + [list(p) for p in ap.ap],
    )


def _body(tc, t):
    nc = tc.nc
    stack = contextlib.ExitStack()
    stack.enter_context(
        nc.allow_low_precision(reason="fp32r/fp16 rounding is intentional; all matmul accumulation stays fp32 in PSUM")
    )
    pool = lambda name, bufs, space="SBUF": stack.enter_context(
        tc.tile_pool(name=name, bufs=bufs, space=space)
    )

    consts = pool("consts", 1)
    singles = pool("singles", 1)
    dramp = pool("dramp", 2, "DRAM")

    # PSUM pools (8 banks): sc 2x[128,1024]=4 | ctx 2 | mmA 2
    ps_sc = pool("ps_sc", 2, "PSUM")    # scores (double-wide) + stats x^2
    ps_ctx = pool("ps_ctx", 1, "PSUM")  # 2 tags: ctx accumulators (head pair)
    ps_mmA = pool("ps_mmA", 2, "PSUM")  # phase A rotation + out-proj + denom bcast

    pA_x = pool("pA_x", 2)      # [128, DC, TB] x block     32KB
    pA_sq = pool("pA_sq", 2)    # [128, TB] squares          4KB
    pA_rows = pool("pA_rows", 2)  # [1, TB] msq/var/lnv      ~8KB
    pA_ge = pool("pA_ge", 1)    # [128, TB] gate tmp         2KB
    pA_vt = pool("pA_vt", 2)    # [128, FQ] v evac tmp       2KB
    pB_pr = pool("pB_pr", 3)    # [128, 2*TB] f16 probs      6KB
    pB_rows = pool("pB_rows", 2)  # [1, TB] recip denom      4KB
    pB_bcs = pool("pB_bcs", 2)  # [64, TB] denom bcast       4KB
    pB_ctxT = pool("pB_ctxT", 2)  # [64, FH, TB] f16 ctx     8KB
    pC_ca = pool("pC_ca", 2)    # [128, TB] f16 ctx_all      2KB
    pC_xr = pool("pC_xr", 1)    # [128, 2, TB] residual      4KB
    pC_osb = pool("pC_osb", 2)  # [128, TB] out staging      4KB

    # constants (fp32r tiles can't be memset directly; stage fp32 + DVE copy)
    onesf_col = consts.tile([128, 1], FP)
    nc.vector.memset(onesf_col, 1.0)
    onesf_row = consts.tile([1, 128], FP)
    nc.vector.memset(onesf_row, 1.0)
    ones_col = consts.tile([128, 1], FR)
    nc.vector.tensor_copy(out=ones_col, in_=onesf_col)
    ones_row = consts.tile([1, 128], FR)
    nc.vector.tensor_copy(out=ones_row, in_=onesf_row)
    eps_t = consts.tile([1, 1], FP)
    nc.vector.memset(eps_t, EPS)

    # resident weights
    wqkg_sb = singles.tile([128, DC, 3 * FQ], FR)
    nc.gpsimd.dma_start(out=wqkg_sb, in_=t["wqkg"].ap().rearrange("(d p) f -> p d f", p=128))
    wv_sb = singles.tile([128, DC, FQ], FR)
    nc.gpsimd.dma_start(out=wv_sb, in_=t["wv"].ap().rearrange("(d p) f -> p d f", p=128))
    wout_sb = singles.tile([128, DC, FQ], F16)
    nc.gpsimd.dma_start(out=wout_sb, in_=t["wout"].ap().rearrange("(d p) f -> p d f", p=128))
    ncs_sb = singles.tile([1, 3 * FQ], FR)
    nc.gpsimd.dma_start(out=ncs_sb, in_=t["ncs"].ap().rearrange("(o f) -> o f", o=1))
    ncsv_sb = singles.tile([1, FQ], FR)
    nc.gpsimd.dma_start(out=ncsv_sb, in_=t["ncsv"].ap().rearrange("(o f) -> o f", o=1))
    bqkg_sb = singles.tile([128, 6], FP)
    nc.sync.dma_start(out=bqkg_sb, in_=t["bqkg"].ap().rearrange("(m p) -> p m", p=128))
    bout_sb = singles.tile([128, 2], FP)
    nc.sync.dma_start(out=bout_sb, in_=t["bout"].ap().rearrange("(m p) -> p m", p=128))
    bv_sb = singles.tile([128, FQ], FP)
    nc.sync.dma_start(out=bv_sb, in_=_bcast_ap(t["bv"], 128))

    # resident activations + per-block LN stats
    qT = singles.tile([128, 2, S], FR)
    kT = singles.tile([128, 2, S], FR)
    gT = singles.tile([128, 2, S], FP)
    va = singles.tile([128, S // 128, FH, HD + 1], F16)  # [k-part, kc, h, 65]
    for _kc in range(S // 128):
        for _h in range(FH):
            nc.vector.tensor_copy(out=va[:, _kc, _h, HD:HD + 1], in_=onesf_col)
    pA_mu = pool("pA_mu", 2)    # [1, TB] FR mean rows (A0(i) -> A1(i))
    pA_rsb = pool("pA_rsb", 2)  # [128, TB] rstd broadcast
    pA_rsc = pool("pA_rsc", 2)  # [128, 4] rstd columns
    mus, rsbs, rscs = {}, {}, {}

    xT_r = t["xT"].ap().rearrange("(d p) tk -> p d tk", p=128)
    xres_r = t["xres"].ap().rearrange("(m p) tk -> p m tk", p=128)

    xblks = {}

    # ---------------- Phase A0: LN stats for one token block --------------
    def phase_a0(i):
        tb = slice(i * TB, (i + 1) * TB)
        xblk = pA_x.tile([128, DC, TB], FR, tag="xblk", name=f"xblk{i}")
        nc.gpsimd.dma_start(out=xblk, in_=xT_r[:, :, tb])
        xblks[i] = xblk

        psx = ps_mmA.tile([1, TB], FP, tag="mmA", name=f"psx{i}")
        for d in range(DC):
            nc.tensor.matmul(
                out=psx, lhsT=ones_col, rhs=xblk[:, d, :],
                start=(d == 0), stop=(d == DC - 1),
            )
        pssq = ps_sc.tile([1, TB], FP, tag="sc", name=f"pssq{i}")
        for d in range(DC):
            xsq = pA_sq.tile([128, TB], FR, tag="xsq", name=f"xsq{i}_{d}")
            nc.vector.tensor_mul(out=xsq, in0=xblk[:, d, :], in1=xblk[:, d, :])
            nc.tensor.matmul(
                out=pssq, lhsT=ones_col, rhs=xsq,
                start=(d == 0), stop=(d == DC - 1),
            )
        mu = pA_mu.tile([1, TB], FR, tag="mu", name=f"mu{i}")
        mus[i] = mu
        nc.scalar.activation(out=mu, in_=psx, func=AF.Copy, scale=1.0 / D)
        msq = pA_rows.tile([1, TB], FP, tag="msq", name=f"msq{i}")
        nc.scalar.activation(out=msq, in_=pssq, func=AF.Copy, scale=1.0 / D)
        var = pA_rows.tile([1, TB], FP, tag="var", name=f"var{i}")
        nc.vector.tensor_mul(out=var, in0=mu, in1=mu)
        nc.vector.tensor_sub(out=var, in0=msq, in1=var)
        # rstd = exp(-0.5 * ln(var + eps))  (keeps everything in one ACT table set)
        lnv = pA_rows.tile([1, TB], FP, tag="lnv", name=f"lnv{i}")
        nc.scalar.activation(out=lnv, in_=var, func=AF.Ln, bias=eps_t[0:1, :])
        rstd = pA_rows.tile([1, TB], FR, tag="rstd", name=f"rstd{i}")
        nc.scalar.activation(out=rstd, in_=lnv, func=AF.Exp, scale=-0.5)
        # broadcast rstd to all partitions (row) and as per-token columns
        rs_b = pA_rsb.tile([128, TB], FP, tag="rs_b", name=f"rsb{i}")
        rsbs[i] = rs_b
        pbc2 = ps_mmA.tile([128, TB], FP, tag="mmA", name=f"pbcrs{i}")
        nc.tensor.matmul(out=pbc2, lhsT=ones_row, rhs=rstd, start=True, stop=True)
        nc.scalar.activation(out=rs_b, in_=pbc2, func=AF.Copy)
        rsc = pA_rsc.tile([128, 4], FR, tag="rsc", name=f"rsc{i}")
        rscs[i] = rsc
        for a in range(4):
            nc.sync.dma_start(
                out=rsc[:, a:a + 1], in_=rstd[0:1, a * 128:(a + 1) * 128]
            )

    # ---------------- Phase A1: projections for one token block -----------
    def phase_a1(i):
        tb = slice(i * TB, (i + 1) * TB)
        xblk = xblks.pop(i)
        mu = mus.pop(i)
        rs_b = rsbs.pop(i)
        rsc = rscs.pop(i)
        # q/k/gate projections on RAW x; mean subtraction folded in as a
        # rank-1 correction (ncs = -colsum(W)); rstd applied at evacuation:
        #   W^T((x-mu)rstd) = rstd * (W^T x + ncs * mu)
        for m in range(6):
            _pl, _tg = (ps_mmA, "mmA") if m % 2 == 0 else (ps_ctx, f"ctxp{m % 4 // 2}")
            pqk = _pl.tile([128, TB], FP, tag=_tg, name=f"pqk{i}_{m}")
            for d in range(DC):
                nc.tensor.matmul(
                    out=pqk,
                    lhsT=wqkg_sb[:, d, m * 128:(m + 1) * 128],
                    rhs=xblk[:, d, :],
                    start=(d == 0), stop=False,
                )
            nc.tensor.matmul(
                out=pqk, lhsT=ncs_sb[0:1, m * 128:(m + 1) * 128], rhs=mu,
                start=False, stop=True,
            )
            if m < 4:
                dst = qT[:, m, tb] if m < 2 else kT[:, m - 2, tb]
                nc.vector.tensor_mul(out=dst, in0=pqk, in1=rs_b)
                nc.vector.tensor_scalar_add(
                    out=dst, in0=dst, scalar1=bqkg_sb[:, m:m + 1]
                )
            else:
                # gate = sigmoid(u + b) = 1 / (1 + exp(-u - b)); bias slot holds -b
                ge = pA_ge.tile([128, TB], FP, tag="ge", name=f"ge{i}_{m}")
                nc.vector.tensor_mul(out=ge, in0=pqk, in1=rs_b)
                nc.scalar.activation(
                    out=ge, in_=ge, func=AF.Exp, scale=-1.0,
                    bias=bqkg_sb[:, m:m + 1],
                )
                nc.vector.tensor_scalar_add(out=ge, in0=ge, scalar1=1.0)
                nc.vector.reciprocal(out=gT[:, m - 4, tb], in_=ge)

        # v projection on RAW x: [tok, feat]; correction mu (x) ncsv; rstd is
        # per-partition (token) at evacuation
        for mt in range(4):
            kcg = i * 4 + mt
            _pl, _tg = (ps_mmA, "mmA") if mt % 2 == 0 else (ps_ctx, f"ctxp{mt // 2}")
            pv = _pl.tile([128, FQ], FP, tag=_tg, name=f"pv{i}_{mt}")
            for d in range(DC):
                nc.tensor.matmul(
                    out=pv,
                    lhsT=xblk[:, d, mt * 128:(mt + 1) * 128],
                    rhs=wv_sb[:, d, :],
                    start=(d == 0), stop=False,
                )
            nc.tensor.matmul(
                out=pv, lhsT=mu[0:1, mt * 128:(mt + 1) * 128], rhs=ncsv_sb,
                start=False, stop=True,
            )
            vtmp = pA_vt.tile([128, FQ], FP, tag="vtmp", name=f"vtmp{i}_{mt}")
            nc.vector.tensor_scalar_mul(
                out=vtmp, in0=pv, scalar1=rsc[:, mt:mt + 1].bitcast(FP)
            )
            for h in range(FH):
                nc.vector.tensor_add(
                    out=va[:, kcg, h, 0:HD],
                    in0=vtmp[:, h * HD:(h + 1) * HD],
                    in1=bv_sb[:, h * HD:(h + 1) * HD],
                )

    # ------- Phase B (attention) / AG / Phase C (output) ------------------
    ctxps = {}
    ctxTs = {}

    def b_pair_chunk(qb, pair, kc0, kc1):
        qs = slice(qb * TB, (qb + 1) * TB)
        if (qb, pair) not in ctxps:
            ctxps[(qb, pair)] = [
                ps_ctx.tile([HD + 1, TB], FP, tag=f"ctxp{j}", name=f"ctxp{qb}_{pair}_{j}")
                for j in range(2)
            ]
        ctxp = ctxps[(qb, pair)]
        for kc in range(kc0, kc1):
            sc = ps_sc.tile([128, 2 * TB], FP, tag="sc", name=f"sc{qb}_{pair}_{kc}")
            for j in range(2):
                nc.tensor.matmul(
                    out=sc[:, j * TB:(j + 1) * TB],
                    lhsT=kT[j * 64:(j + 1) * 64, pair, kc * 128:(kc + 1) * 128],
                    rhs=qT[j * 64:(j + 1) * 64, pair, qs],
                    start=True, stop=True, skip_group_check=True,
                )
            pr = pB_pr.tile([128, 2 * TB], F16, tag="pr", name=f"pr{qb}_{pair}_{kc}")
            nc.scalar.activation(out=pr, in_=sc, func=AF.Exp)
            for j in range(2):
                h = 2 * pair + j
                nc.tensor.matmul(
                    out=ctxp[j],
                    lhsT=va[:, kc, h, :],
                    rhs=pr[:, j * TB:(j + 1) * TB],
                    start=(kc == 0), stop=(kc == S // 128 - 1),
                )

    def b_pair_finish(qb, pair):
        if qb not in ctxTs:
            ctxTs[qb] = pB_ctxT.tile([64, FH, TB], F16, tag="ctxT", name=f"ctxT{qb}")
        ctxT = ctxTs[qb]
        ctxp = ctxps.pop((qb, pair))
        for j in range(2):
            h = 2 * pair + j
            rden = pB_rows.tile([1, TB], FR, tag="rden", name=f"rden{qb}_{h}")
            nc.vector.reciprocal(out=rden, in_=ctxp[j][HD:HD + 1, :])
            bcp = ps_mmA.tile([64, TB], FP, tag="mmA", name=f"bcp{qb}_{h}")
            nc.tensor.matmul(
                out=bcp, lhsT=ones_row[0:1, 0:64], rhs=rden, start=True, stop=True
            )
            bcs = pB_bcs.tile([64, TB], FP, tag="bcs", name=f"bcs{qb}_{h}")
            nc.vector.tensor_copy(out=bcs, in_=bcp)
            nc.vector.tensor_mul(
                out=ctxT[:, h, :], in0=ctxp[j][0:HD, :], in1=bcs
            )

    def b_finish(qb):
        ctxT = ctxTs.pop(qb)
        cin = dramp.tile([FQ, TB], F16, tag="cin", name=f"cin{qb}")
        for h in range(FH):
            nc.sync.dma_start(out=cin[h * HD:(h + 1) * HD, :], in_=ctxT[:, h, :])
        call = dramp.tile([4, FQ, TB], F16, tag="call", name=f"call{qb}")
        nc.gpsimd.collective_compute(
            "AllGather",
            mybir.AluOpType.bypass,
            replica_groups=GROUPS,
            ins=[cin.opt()],
            outs=[call.opt()],
        )
        return call

    def phase_b(qb):
        b_pair_chunk(qb, 0, 0, S // 128)
        b_pair_finish(qb, 0)
        b_pair_chunk(qb, 1, 0, S // 128)
        b_pair_finish(qb, 1)
        return b_finish(qb)

    def phase_c(qb, call):
        qs = slice(qb * TB, (qb + 1) * TB)
        xres_sb = pC_xr.tile([128, 2, TB], FP, tag="xres_sb", name=f"xres{qb}")
        nc.sync.dma_start(out=xres_sb, in_=xres_r[:, :, qs])
        po = [
            ps_mmA.tile([128, TB], FP, tag="mmA", name=f"po{qb}_{m}")
            for m in range(2)
        ]
        for c8 in range(DC):
            g4, r = divmod(c8, 2)
            ca = pC_ca.tile([128, TB], F16, tag="ca", name=f"ca{qb}_{c8}")
            nc.sync.dma_start(out=ca, in_=call[g4, r * 128:(r + 1) * 128, :])
            for m in range(2):
                nc.tensor.matmul(
                    out=po[m],
                    lhsT=wout_sb[:, c8, m * 128:(m + 1) * 128],
                    rhs=ca,
                    start=(c8 == 0), stop=(c8 == DC - 1),
                )
        for m in range(2):
            osb = pC_osb.tile([128, TB], FP, tag="osb", name=f"osb{qb}_{m}")
            nc.vector.tensor_scalar_add(out=osb, in0=po[m], scalar1=bout_sb[:, m:m + 1])
            nc.vector.tensor_mul(out=osb, in0=osb, in1=gT[:, m, qs])
            nc.vector.tensor_add(out=osb, in0=osb, in1=xres_sb[:, m, :])
            nc.sync.dma_start(out=t["outT"].ap()[m * 128:(m + 1) * 128, qs], in_=osb)

    # software-pipelined emission: B(0) pair 0 is emitted chunk-by-chunk
    # right after the A1 block that produces its k/v tiles, hiding its exp
    # work under the (ACT-idle) projection phase
    calls = {}
    phase_a0(0)
    phase_a0(1)
    phase_a1(0)
    phase_a0(2)
    phase_a1(1)
    phase_a0(3)
    phase_a1(2)
    phase_a1(3)
    calls[0] = phase_b(0)
    calls[1] = phase_b(1)
    phase_c(0, calls[0])
    calls[2] = phase_b(2)
    phase_c(1, calls[1])
    calls[3] = phase_b(3)
    phase_c(2, calls[2])
    phase_c(3, calls[3])

    stack.close()


def build_nc():
    if "nc" in _NC_CACHE:
        return _NC_CACHE["nc"]
    nc = bacc.Bacc("TRN2", target_bir_lowering=False, debug=False, num_devices=NCORES)
    t = {}
    t["xT"] = nc.dram_tensor("xT", [D, S], FP, kind="ExternalInput")
    t["xres"] = nc.dram_tensor("xres", [FQ, S], FP, kind="ExternalInput")
    t["wqkg"] = nc.dram_tensor("wqkg", [D, 3 * FQ], FP, kind="ExternalInput")
    t["wv"] = nc.dram_tensor("wv", [D, FQ], FP, kind="ExternalInput")
    t["wout"] = nc.dram_tensor("wout", [D, FQ], FP, kind="ExternalInput")
    t["bqkg"] = nc.dram_tensor("bqkg", [3 * FQ], FP, kind="ExternalInput")
    t["ncs"] = nc.dram_tensor("ncs", [3 * FQ], FP, kind="ExternalInput")
    t["ncsv"] = nc.dram_tensor("ncsv", [FQ], FP, kind="ExternalInput")
    t["bv"] = nc.dram_tensor("bv", [FQ], FP, kind="ExternalInput")
    t["bout"] = nc.dram_tensor("bout", [FQ], FP, kind="ExternalInput")
    t["outT"] = nc.dram_tensor("outT", [FQ, S], FP, kind="ExternalOutput")
    with tile.TileContext(nc) as tc:
        _body(tc, t)
    nc.finalize()
    _NC_CACHE["nc"] = nc
    return nc


def make_in_maps(x, gamma, beta, w_qkv, b_qkv, w_out, b_out, w_gate, b_gate):
    x = np.asarray(x, np.float32)
    gamma = np.asarray(gamma, np.float32)
    beta = np.asarray(beta, np.float32)
    w_qkv = np.asarray(w_qkv, np.float32)
    b_qkv = np.asarray(b_qkv, np.float32)
    w_out = np.asarray(w_out, np.float32)
    b_out = np.asarray(b_out, np.float32)
    w_gate = np.asarray(w_gate, np.float32)
    b_gate = np.asarray(b_gate, np.float32)

    scale = np.float32(1.0 / np.sqrt(HD))
    xT = [np.ascontiguousarray(x[b].T) for b in range(B)]
    in_maps = []
    for c in range(NCORES):
        b, g = divmod(c, 4)
        cols = slice(g * FQ, (g + 1) * FQ)
        wq = w_qkv[:, 0 * D:1 * D][:, cols]
        wk = w_qkv[:, 1 * D:2 * D][:, cols]
        wv = w_qkv[:, 2 * D:3 * D][:, cols]
        bq = b_qkv[0 * D:1 * D][cols]
        bk = b_qkv[1 * D:2 * D][cols]
        bv = b_qkv[2 * D:3 * D][cols]
        wg = w_gate[:, cols]
        bg = b_gate[cols]

        gfold = lambda w: gamma[:, None] * w
        bfold = lambda w, bb: bb + beta @ w

        wq_e = gfold(wq) * scale
        bq_e = bfold(wq, bq) * scale
        wk_e = gfold(wk)
        bk_e = bfold(wk, bk)
        wv_e = gfold(wv)
        bv_e = bfold(wv, bv)
        wg_e = gfold(wg)
        bg_e = -bfold(wg, bg)  # negated: used as bias of exp(-u - b)

        in_maps.append({
            "xT": xT[b],
            "xres": np.ascontiguousarray(xT[b][cols, :]),
            "wqkg": np.ascontiguousarray(
                np.concatenate([wq_e, wk_e, wg_e], axis=1).astype(np.float32)
            ),
            "ncs": -np.concatenate([wq_e, wk_e, wg_e], axis=1).sum(axis=0).astype(np.float32),
            "ncsv": -wv_e.sum(axis=0).astype(np.float32),
            "wv": np.ascontiguousarray(wv_e.astype(np.float32)),
            "wout": np.ascontiguousarray(w_out[:, cols]),
            "bqkg": np.concatenate([bq_e, bk_e, bg_e]).astype(np.float32),
            "bv": bv_e.astype(np.float32),
            "bout": np.ascontiguousarray(b_out[cols]),
        })
    return in_maps


def run_device(in_maps):
    nc = build_nc()
    return run_bass_kernel_spmd(nc, in_maps, list(range(NCORES)))


def assemble(results):
    out = np.empty((B, S, D), np.float32)
    for c in range(NCORES):
        b, g = divmod(c, 4)
        out[b][:, g * FQ:(g + 1) * FQ] = results[c]["outT"].T
    return out


def kernel(**inputs):
    in_maps = make_in_maps(**inputs)
    res = run_device(in_maps)
    return assemble(res.results)


# revision 21
# speedup vs baseline: 1.2432x; 1.0355x over previous
"""EnhancedMultiHeadAttention on 8 Trainium2 NeuronCores (Bass/Tile).

Sharding: core c -> batch b = c//4, head group g = c%4 (4 heads of 16).
Per core, everything is computed in "transposed" layout [feature, token]:
  - LayerNorm stats via ones-matmul column sums of x and x^2 (PE), then
    normalize xT in place with broadcast mu/rstd rows (K=1 ones matmuls).
  - Fused projections q/k/gate in [feat, tok] layout; v in [tok, feat]
    layout (lhsT = zT) augmented with a ones column per head so the
    attention AV matmul also produces the softmax denominator.
  - Scores^T via lhsT=kT slice, rhs=qT slice (K=HD=64, head pairs packed
    into PE row groups 0-63/64-127); softmax over k is a plain exp (no
    max subtraction; scores are provably small for this model) and the
    denominator lands in ctx PSUM row 64 via the V ones column.
  - ctx rows are normalized by 1/denominator (broadcast via K=1 matmul),
    AllGather'd across the 4 cores of the batch group per q-block, then
    out = ctx_all @ w_out[:, cols] + b_out, gated and residual-added.
All LayerNorm gamma/beta and the 1/sqrt(HD) scale are folded into the
weights/biases on the host. sigmoid and rsqrt are computed via exp/ln so
the whole kernel uses one ACT table set (natural_log_exp_and_others).
"""

import contextlib
import os

import numpy as np

import jax

jax.config.update("jax_compilation_cache_dir", os.path.expanduser("~/.bass_jax_cache"))
jax.config.update("jax_persistent_cache_min_compile_time_secs", 0.0)
jax.config.update("jax_persistent_cache_min_entry_size_bytes", 0)

import concourse.bass as bass
import concourse.bacc as bacc
import concourse.tile as tile
from concourse import mybir
from concourse.bass_utils import run_bass_kernel_spmd
from concourse.hw_specs import get_activation_tables as _orig_gat


def _patched_gat(arch):
    # Steer the greedy ACT-table chooser to the combined ln+exp set so the
    # kernel needs exactly one table load instead of thrashing between
    # exp_and_others and natural_log every block (~2.7us per reload).
    tabs = {k: set(v) for k, v in _orig_gat(arch).items()}
    _AF = mybir.ActivationFunctionType
    for nm in ("exp_and_others", "exp_and_friends"):
        if nm in tabs:
            tabs[nm].discard(_AF.Exp)
    if "natural_log" in tabs:
        tabs["natural_log"].discard(_AF.Ln)
    return tabs


bacc.get_activation_tables = _patched_gat

B, S, D, H, HD = 2, 2048, 1024, 16, 64
NCORES = 8
GROUPS = [[0, 1, 2, 3], [4, 5, 6, 7]]
TB = 512  # token block
NB = S // TB  # 4
DC = D // 128  # 8 K-chunks
FH = 4  # heads per core
FQ = FH * HD  # 256 feature columns per core
FP = mybir.dt.float32
FR = mybir.dt.float32r  # TF32-like: 4x matmul throughput vs fp32
F16 = mybir.dt.float16  # halves AllGather bytes; ~5e-4 elementwise rounding
AF = mybir.ActivationFunctionType
EPS = 1e-5

_NC_CACHE = {}


def _bcast_ap(handle, parts):
    ap = handle.ap()
    return bass.AP(
        tensor=ap.tensor,
        offset=ap.offset,
        ap=[[0, parts]] + [list(p) for p in ap.ap],
    )


def _body(tc, t):
    nc = tc.nc
    stack = contextlib.ExitStack()
    stack.enter_context(
        nc.allow_low_precision(reason="fp32r/fp16 rounding is intentional; all matmul accumulation stays fp32 in PSUM")
    )
    pool = lambda name, bufs, space="SBUF": stack.enter_context(
        tc.tile_pool(name=name, bufs=bufs, space=space)
    )

    consts = pool("consts", 1)
    singles = pool("singles", 1)
    dramp = pool("dramp", 2, "DRAM")

    # PSUM pools (8 banks): sc 2x[128,1024]=4 | ctx 2 | mmA 2
    ps_sc = pool("ps_sc", 2, "PSUM")    # scores (double-wide) + stats x^2
    ps_ctx = pool("ps_ctx", 1, "PSUM")  # 2 tags: ctx accumulators (head pair)
    ps_mmA = pool("ps_mmA", 2, "PSUM")  # phase A rotation + out-proj + denom bcast

    pA_x = pool("pA_x", 2)      # [128, DC, TB] x block     32KB
    pA_sq = pool("pA_sq", 2)    # [128, TB] squares          4KB
    pA_rows = pool("pA_rows", 2)  # [1, TB] msq/var/lnv      ~8KB
    pA_ge = pool("pA_ge", 1)    # [128, TB] gate tmp         2KB
    pA_vt = pool("pA_vt", 2)    # [128, FQ] v evac tmp       2KB
    pB_pr = pool("pB_pr", 3)    # [128, 2*TB] f16 probs      6KB
    pB_rows = pool("pB_rows", 2)  # [1, TB] recip denom      4KB
    pB_bcs = pool("pB_bcs", 2)  # [64, TB] denom bcast       4KB
    pB_ctxT = pool("pB_ctxT", 2)  # [64, FH, TB] f16 ctx     8KB
    pC_ca = pool("pC_ca", 2)    # [128, TB] f16 ctx_all      2KB
    pC_xr = pool("pC_xr", 1)    # [128, 2, TB] residual      4KB
    pC_osb = pool("pC_osb", 2)  # [128, TB] out staging      4KB

    # constants (fp32r tiles can't be memset directly; stage fp32 + DVE copy)
    onesf_col = consts.tile([128, 1], FP)
    nc.vector.memset(onesf_col, 1.0)
    onesf_row = consts.tile([1, 128], FP)
    nc.vector.memset(onesf_row, 1.0)
    ones_col = consts.tile([128, 1], FR)
    nc.vector.tensor_copy(out=ones_col, in_=onesf_col)
    ones_row = consts.tile([1, 128], FR)
    nc.vector.tensor_copy(out=ones_row, in_=onesf_row)
    eps_t = consts.tile([1, 1], FP)
    nc.vector.memset(eps_t, EPS)

    # resident weights
    wqkg_sb = singles.tile([128, DC, 3 * FQ], FR)
    nc.gpsimd.dma_start(out=wqkg_sb, in_=t["wqkg"].ap().rearrange("(d p) f -> p d f", p=128))
    wv_sb = singles.tile([128, DC, FQ], FR)
    nc.gpsimd.dma_start(out=wv_sb, in_=t["wv"].ap().rearrange("(d p) f -> p d f", p=128))
    wout_sb = singles.tile([128, DC, FQ], F16)
    nc.gpsimd.dma_start(out=wout_sb, in_=t["wout"].ap().rearrange("(d p) f -> p d f", p=128))
    ncs_sb = singles.tile([1, 3 * FQ], FR)
    nc.gpsimd.dma_start(out=ncs_sb, in_=t["ncs"].ap().rearrange("(o f) -> o f", o=1))
    ncsv_sb = singles.tile([1, FQ], FR)
    nc.gpsimd.dma_start(out=ncsv_sb, in_=t["ncsv"].ap().rearrange("(o f) -> o f", o=1))
    bqkg_sb = singles.tile([128, 6], FP)
    nc.sync.dma_start(out=bqkg_sb, in_=t["bqkg"].ap().rearrange("(m p) -> p m", p=128))
    bout_sb = singles.tile([128, 2], FP)
    nc.sync.dma_start(out=bout_sb, in_=t["bout"].ap().rearrange("(m p) -> p m", p=128))
    bv_sb = singles.tile([128, FQ], FP)
    nc.sync.dma_start(out=bv_sb, in_=_bcast_ap(t["bv"], 128))

    # resident activations + per-block LN stats
    qT = singles.tile([128, 2, S], FR)
    kT = singles.tile([128, 2, S], FR)
    gT = singles.tile([128, 2, S], FP)
    va = singles.tile([128, S // 128, FH, HD + 1], F16)  # [k-part, kc, h, 65]
    for _kc in range(S // 128):
        for _h in range(FH):
            nc.vector.tensor_copy(out=va[:, _kc, _h, HD:HD + 1], in_=onesf_col)
    pA_mu = pool("pA_mu", 2)    # [1, TB] FR mean rows (A0(i) -> A1(i))
    pA_rsb = pool("pA_rsb", 2)  # [128, TB] rstd broadcast
    pA_rsc = pool("pA_rsc", 2)  # [128, 4] rstd columns
    mus, rsbs, rscs = {}, {}, {}

    xT_r = t["xT"].ap().rearrange("(d p) tk -> p d tk", p=128)
    xres_r = t["xres"].ap().rearrange("(m p) tk -> p m tk", p=128)

    xblks = {}

    # ---------------- Phase A0: LN stats for one token block --------------
    def phase_a0(i):
        tb = slice(i * TB, (i + 1) * TB)
        xblk = pA_x.tile([128, DC, TB], FR, tag="xblk", name=f"xblk{i}")
        nc.gpsimd.dma_start(out=xblk, in_=xT_r[:, :, tb])
        xblks[i] = xblk

        psx = ps_mmA.tile([1, TB], FP, tag="mmA", name=f"psx{i}")
        for d in range(DC):
            nc.tensor.matmul(
                out=psx, lhsT=ones_col, rhs=xblk[:, d, :],
                start=(d == 0), stop=(d == DC - 1),
            )
        pssq = ps_sc.tile([1, TB], FP, tag="sc", name=f"pssq{i}")
        for d in range(DC):
            xsq = pA_sq.tile([128, TB], FR, tag="xsq", name=f"xsq{i}_{d}")
            nc.vector.tensor_mul(out=xsq, in0=xblk[:, d, :], in1=xblk[:, d, :])
            nc.tensor.matmul(
                out=pssq, lhsT=ones_col, rhs=xsq,
                start=(d == 0), stop=(d == DC - 1),
            )
        mu = pA_mu.tile([1, TB], FR, tag="mu", name=f"mu{i}")
        mus[i] = mu
        nc.scalar.activation(out=mu, in_=psx, func=AF.Copy, scale=1.0 / D)
        msq = pA_rows.tile([1, TB], FP, tag="msq", name=f"msq{i}")
        nc.scalar.activation(out=msq, in_=pssq, func=AF.Copy, scale=1.0 / D)
        var = pA_rows.tile([1, TB], FP, tag="var", name=f"var{i}")
        nc.vector.tensor_mul(out=var, in0=mu, in1=mu)
        nc.vector.tensor_sub(out=var, in0=msq, in1=var)
        # rstd = exp(-0.5 * ln(var + eps))  (keeps everything in one ACT table set)
        lnv = pA_rows.tile([1, TB], FP, tag="lnv", name=f"lnv{i}")
        nc.scalar.activation(out=lnv, in_=var, func=AF.Ln, bias=eps_t[0:1, :])
        rstd = pA_rows.tile([1, TB], FR, tag="rstd", name=f"rstd{i}")
        nc.scalar.activation(out=rstd, in_=lnv, func=AF.Exp, scale=-0.5)
        # broadcast rstd to all partitions (row) and as per-token columns
        rs_b = pA_rsb.tile([128, TB], FP, tag="rs_b", name=f"rsb{i}")
        rsbs[i] = rs_b
        pbc2 = ps_mmA.tile([128, TB], FP, tag="mmA", name=f"pbcrs{i}")
        nc.tensor.matmul(out=pbc2, lhsT=ones_row, rhs=rstd, start=True, stop=True)
        nc.scalar.activation(out=rs_b, in_=pbc2, func=AF.Copy)
        rsc = pA_rsc.tile([128, 4], FR, tag="rsc", name=f"rsc{i}")
        rscs[i] = rsc
        for a in range(4):
            nc.sync.dma_start(
                out=rsc[:, a:a + 1], in_=rstd[0:1, a * 128:(a + 1) * 128]
            )

    # ---------------- Phase A1: projections for one token block -----------
    def phase_a1(i):
        tb = slice(i * TB, (i + 1) * TB)
        xblk = xblks.pop(i)
        mu = mus.pop(i)
        rs_b = rsbs.pop(i)
        rsc = rscs.pop(i)
        # q/k/gate projections on RAW x; mean subtraction folded in as a
        # rank-1 correction (ncs = -colsum(W)); rstd applied at evacuation:
        #   W^T((x-mu)rstd) = rstd * (W^T x + ncs * mu)
        for m in range(6):
            _pl, _tg = (ps_mmA, "mmA") if m % 2 == 0 else (ps_ctx, f"ctxp{m % 4 // 2}")
            pqk = _pl.tile([128, TB], FP, tag=_tg, name=f"pqk{i}_{m}")
            for d in range(DC):
                nc.tensor.matmul(
                    out=pqk,
                    lhsT=wqkg_sb[:, d, m * 128:(m + 1) * 128],
                    rhs=xblk[:, d, :],
                    start=(d == 0), stop=False,
                )
            nc.tensor.matmul(
                out=pqk, lhsT=ncs_sb[0:1, m * 128:(m + 1) * 128], rhs=mu,
                start=False, stop=True,
            )
            if m < 4:
                dst = qT[:, m, tb] if m < 2 else kT[:, m - 2, tb]
                nc.vector.tensor_mul(out=dst, in0=pqk, in1=rs_b)
                nc.vector.tensor_scalar_add(
                    out=dst, in0=dst, scalar1=bqkg_sb[:, m:m + 1]
                )
            else:
                # gate = sigmoid(u + b) = 1 / (1 + exp(-u - b)); bias slot holds -b
                ge = pA_ge.tile([128, TB], FP, tag="ge", name=f"ge{i}_{m}")
                nc.vector.tensor_mul(out=ge, in0=pqk, in1=rs_b)
                nc.scalar.activation(
                    out=ge, in_=ge, func=AF.Exp, scale=-1.0,
                    bias=bqkg_sb[:, m:m + 1],
                )
                nc.vector.tensor_scalar_add(out=ge, in0=ge, scalar1=1.0)
                nc.vector.reciprocal(out=gT[:, m - 4, tb], in_=ge)

        # v projection on RAW x: [tok, feat]; correction mu (x) ncsv; rstd is
        # per-partition (token) at evacuation
        for mt in range(4):
            kcg = i * 4 + mt
            _pl, _tg = (ps_mmA, "mmA") if mt % 2 == 0 else (ps_ctx, f"ctxp{mt // 2}")
            pv = _pl.tile([128, FQ], FP, tag=_tg, name=f"pv{i}_{mt}")
            for d in range(DC):
                nc.tensor.matmul(
                    out=pv,
                    lhsT=xblk[:, d, mt * 128:(mt + 1) * 128],
                    rhs=wv_sb[:, d, :],
                    start=(d == 0), stop=False,
                )
            nc.tensor.matmul(
                out=pv, lhsT=mu[0:1, mt * 128:(mt + 1) * 128], rhs=ncsv_sb,
                start=False, stop=True,
            )
            vtmp = pA_vt.tile([128, FQ], FP, tag="vtmp", name=f"vtmp{i}_{mt}")
            nc.vector.tensor_scalar_mul(
                out=vtmp, in0=pv, scalar1=rsc[:, mt:mt + 1].bitcast(FP)
            )
            for h in range(FH):
                nc.vector.tensor_add(
                    out=va[:, kcg, h, 0:HD],
                    in0=vtmp[:, h * HD:(h + 1) * HD],
                    in1=bv_sb[:, h * HD:(h + 1) * HD],
                )

    # ------- Phase B (attention) / AG / Phase C (output) ------------------
    ctxps = {}
    ctxTs = {}

    def b_pair_chunk(qb, pair, kc0, kc1):
        qs = slice(qb * TB, (qb + 1) * TB)
        if (qb, pair) not in ctxps:
            ctxps[(qb, pair)] = [
                ps_ctx.tile([HD + 1, TB], FP, tag=f"ctxp{j}", name=f"ctxp{qb}_{pair}_{j}")
                for j in range(2)
            ]
        ctxp = ctxps[(qb, pair)]
        for kc in range(kc0, kc1):
            sc = ps_sc.tile([128, 2 * TB], FP, tag="sc", name=f"sc{qb}_{pair}_{kc}")
            for j in range(2):
                nc.tensor.matmul(
                    out=sc[:, j * TB:(j + 1) * TB],
                    lhsT=kT[j * 64:(j + 1) * 64, pair, kc * 128:(kc + 1) * 128],
                    rhs=qT[j * 64:(j + 1) * 64, pair, qs],
                    start=True, stop=True, skip_group_check=True,
                )
            pr = pB_pr.tile([128, 2 * TB], F16, tag="pr", name=f"pr{qb}_{pair}_{kc}")
            nc.scalar.activation(out=pr, in_=sc, func=AF.Exp)
            for j in range(2):
                h = 2 * pair + j
                nc.tensor.matmul(
                    out=ctxp[j],
                    lhsT=va[:, kc, h, :],
                    rhs=pr[:, j * TB:(j + 1) * TB],
                    start=(kc == 0), stop=(kc == S // 128 - 1),
                )

    def b_pair_finish(qb, pair):
        if qb not in ctxTs:
            ctxTs[qb] = pB_ctxT.tile([64, FH, TB], F16, tag="ctxT", name=f"ctxT{qb}")
        ctxT = ctxTs[qb]
        ctxp = ctxps.pop((qb, pair))
        for j in range(2):
            h = 2 * pair + j
            rden = pB_rows.tile([1, TB], FR, tag="rden", name=f"rden{qb}_{h}")
            nc.vector.reciprocal(out=rden, in_=ctxp[j][HD:HD + 1, :])
            bcp = ps_mmA.tile([64, TB], FP, tag="mmA", name=f"bcp{qb}_{h}")
            nc.tensor.matmul(
                out=bcp, lhsT=ones_row[0:1, 0:64], rhs=rden, start=True, stop=True
            )
            bcs = pB_bcs.tile([64, TB], FP, tag="bcs", name=f"bcs{qb}_{h}")
            nc.vector.tensor_copy(out=bcs, in_=bcp)
            nc.vector.tensor_mul(
                out=ctxT[:, h, :], in0=ctxp[j][0:HD, :], in1=bcs
            )

    def b_finish(qb):
        ctxT = ctxTs.pop(qb)
        cin = dramp.tile([FQ, TB], F16, tag="cin", name=f"cin{qb}")
        for h in range(FH):
            nc.sync.dma_start(out=cin[h * HD:(h + 1) * HD, :], in_=ctxT[:, h, :])
        call = dramp.tile([4, FQ, TB], F16, tag="call", name=f"call{qb}")
        nc.gpsimd.collective_compute(
            "AllGather",
            mybir.AluOpType.bypass,
            replica_groups=GROUPS,
            ins=[cin.opt()],
            outs=[call.opt()],
        )
        return call

    def phase_b(qb):
        b_pair_chunk(qb, 0, 0, S // 128)
        b_pair_finish(qb, 0)
        b_pair_chunk(qb, 1, 0, S // 128)
        b_pair_finish(qb, 1)
        return b_finish(qb)

    def phase_c(qb, call):
        qs = slice(qb * TB, (qb + 1) * TB)
        xres_sb = pC_xr.tile([128, 2, TB], FP, tag="xres_sb", name=f"xres{qb}")
        nc.sync.dma_start(out=xres_sb, in_=xres_r[:, :, qs])
        po = [
            ps_mmA.tile([128, TB], FP, tag="mmA", name=f"po{qb}_{m}")
            for m in range(2)
        ]
        for c8 in range(DC):
            g4, r = divmod(c8, 2)
            ca = pC_ca.tile([128, TB], F16, tag="ca", name=f"ca{qb}_{c8}")
            nc.sync.dma_start(out=ca, in_=call[g4, r * 128:(r + 1) * 128, :])
            for m in range(2):
                nc.tensor.matmul(
                    out=po[m],
                    lhsT=wout_sb[:, c8, m * 128:(m + 1) * 128],
                    rhs=ca,
                    start=(c8 == 0), stop=(c8 == DC - 1),
                )
        for m in range(2):
            osb = pC_osb.tile([128, TB], FP, tag="osb", name=f"osb{qb}_{m}")
            nc.vector.tensor_scalar_add(out=osb, in0=po[m], scalar1=bout_sb[:, m:m + 1])
            nc.vector.tensor_mul(out=osb, in0=osb, in1=gT[:, m, qs])
            nc.vector.tensor_add(out=osb, in0=osb, in1=xres_sb[:, m, :])
            nc.sync.dma_start(out=t["outT"].ap()[m * 128:(m + 1) * 128, qs], in_=osb)

    # software-pipelined emission: B(0) pair 0 is emitted chunk-by-chunk
    # right after the A1 block that produces its k/v tiles, hiding its exp
    # work under the (ACT-idle) projection phase
    calls = {}
    phase_a0(0)
    phase_a0(1)
    phase_a1(0)
    phase_a0(2)
    phase_a1(1)
    phase_a0(3)
    phase_a1(2)
    phase_a1(3)
    calls[0] = phase_b(0)
    calls[1] = phase_b(1)
    phase_c(0, calls[0])
    calls[2] = phase_b(2)
    phase_c(1, calls[1])
    calls[3] = phase_b(3)
    phase_c(2, calls[2])
    phase_c(3, calls[3])

    stack.close()


def build_nc():
    if "nc" in _NC_CACHE:
        return _NC_CACHE["nc"]
    nc = bacc.Bacc("TRN2", target_bir_lowering=False, debug=False, num_devices=NCORES)
    t = {}
    t["xT"] = nc.dram_tensor("xT", [D, S], FP, kind="ExternalInput")
    t["xres"] = nc.dram_tensor("xres", [FQ, S], FP, kind="ExternalInput")
    t["wqkg"] = nc.dram_tensor("wqkg", [D, 3 * FQ], FP, kind="ExternalInput")
    t["wv"] = nc.dram_tensor("wv", [D, FQ], FP, kind="ExternalInput")
    t["wout"] = nc.dram_tensor("wout", [D, FQ], FP, kind="ExternalInput")
    t["bqkg"] = nc.dram_tensor("bqkg", [3 * FQ], FP, kind="ExternalInput")
    t["ncs"] = nc.dram_tensor("ncs", [3 * FQ], FP, kind="ExternalInput")
    t["ncsv"] = nc.dram_tensor("ncsv", [FQ], FP, kind="ExternalInput")
    t["bv"] = nc.dram_tensor("bv", [FQ], FP, kind="ExternalInput")
    t["bout"] = nc.dram_tensor("bout", [FQ], FP, kind="ExternalInput")
    t["outT"] = nc.dram_tensor("outT", [FQ, S], FP, kind="ExternalOutput")
    with tile.TileContext(nc) as tc:
        _body(tc, t)
    nc.finalize()
    _NC_CACHE["nc"] = nc
    return nc


def make_in_maps(x, gamma, beta, w_qkv, b_qkv, w_out, b_out, w_gate, b_gate):
    x = np.asarray(x, np.float32)
    gamma = np.asarray(gamma, np.float32)
    beta = np.asarray(beta, np.float32)
    w_qkv = np.asarray(w_qkv, np.float32)
    b_qkv = np.asarray(b_qkv, np.float32)
    w_out = np.asarray(w_out, np.float32)
    b_out = np.asarray(b_out, np.float32)
    w_gate = np.asarray(w_gate, np.float32)
    b_gate = np.asarray(b_gate, np.float32)

    scale = np.float32(1.0 / np.sqrt(HD))
    xT = [np.ascontiguousarray(x[b].T) for b in range(B)]
    in_maps = []
    for c in range(NCORES):
        b, g = divmod(c, 4)
        cols = slice(g * FQ, (g + 1) * FQ)
        wq = w_qkv[:, 0 * D:1 * D][:, cols]
        wk = w_qkv[:, 1 * D:2 * D][:, cols]
        wv = w_qkv[:, 2 * D:3 * D][:, cols]
        bq = b_qkv[0 * D:1 * D][cols]
        bk = b_qkv[1 * D:2 * D][cols]
        bv = b_qkv[2 * D:3 * D][cols]
        wg = w_gate[:, cols]
        bg = b_gate[cols]

        gfold = lambda w: gamma[:, None] * w
        bfold = lambda w, bb: bb + beta @ w

        wq_e = gfold(wq) * scale
        bq_e = bfold(wq, bq) * scale
        wk_e = gfold(wk)
        bk_e = bfold(wk, bk)
        wv_e = gfold(wv)
        bv_e = bfold(wv, bv)
        wg_e = gfold(wg)
        bg_e = -bfold(wg, bg)  # negated: used as bias of exp(-u - b)

        in_maps.append({
            "xT": xT[b],
            "xres": np.ascontiguousarray(xT[b][cols, :]),
            "wqkg": np.ascontiguousarray(
                np.concatenate([wq_e, wk_e, wg_e], axis=1).astype(np.float32)
            ),
            "ncs": -np.concatenate([wq_e, wk_e, wg_e], axis=1).sum(axis=0).astype(np.float32),
            "ncsv": -wv_e.sum(axis=0).astype(np.float32),
            "wv": np.ascontiguousarray(wv_e.astype(np.float32)),
            "wout": np.ascontiguousarray(w_out[:, cols]),
            "bqkg": np.concatenate([bq_e, bk_e, bg_e]).astype(np.float32),
            "bv": bv_e.astype(np.float32),
            "bout": np.ascontiguousarray(b_out[cols]),
        })
    return in_maps


def run_device(in_maps):
    nc = build_nc()
    return run_bass_kernel_spmd(nc, in_maps, list(range(NCORES)))


def assemble(results):
    out = np.empty((B, S, D), np.float32)
    for c in range(NCORES):
        b, g = divmod(c, 4)
        out[b][:, g * FQ:(g + 1) * FQ] = results[c]["outT"].T
    return out


def kernel(**inputs):
    in_maps = make_in_maps(**inputs)
    res = run_device(in_maps)
    return assemble(res.results)
